# revision 10
# baseline (speedup 1.0000x reference)
# Lattice gauge CNN (L-CNN) layer on 8 Trainium2 NeuronCores via Bass/Tile.
#
# Reference computation per batch element (B=8 -> one per core):
#   W = concat(plaquettes(U) [6ch], polyakov(U) [4ch])      # [L^4, 10, 3, 3] c64
#   out[i] = sum_m U_m ( sum_{j,k} omega[i,j,m,k] roll(W_j, -k, m) ) U_m^dag
#
# Per-core dataflow (complex math decomposed into re/im fp32 planes):
#   site layout: p = x1*9 + (x2//4)*3 + (x3//4)  in [0,108)  (SBUF partitions)
#                f = (x3%4)*48 + (x2%4)*12 + x4  in [0,192)  (SBUF free)
#   DRAM site index s = p*192 + f ("layout order"; host pre-permutes).
#   Phase A:  per-site 3x3 complex matmul chains on DVE/GPSIMD -> w_dram
#   Phase C1: channel mix (omega) as TensorE matmuls, cols=(elem,site) -> z_dram
#   Phase C2: rolled loads of z + adds -> S tiles (per direction m)
#   Phase D:  sandwich U_m S U_m^dag on DVE/GPSIMD -> out_dram
#
# Lattice rolls = piecewise moves: each roll decomposes into <=4 boxes with
# constant (partition-delta, free-delta). Partition-crossing pieces become
# contiguous-partition-window DMAs; pure-free pieces become engine copies.

import numpy as np

L = 12
NS = L ** 4          # 20736 sites per batch element
NP = 108             # SBUF partitions used
NF = 192             # free site dim
N_OUT = 8

# sub-dims: name -> (radix, p-stride or None, f-stride or None)
# x2 = l2*4 + h2 ; x3 = l3*4 + h3 ; p = x1*9+l2*3+l3 ; f = h3*48+h2*12+x4
SUB = {
    "x1": (12, 9, None),
    "l2": (3, 3, None),
    "h2": (4, None, 12),
    "l3": (3, 1, None),
    "h3": (4, None, 48),
    "x4": (12, None, 1),
}
AXIS_SUBS = {1: ("x1",), 2: ("l2", "h2"), 3: ("l3", "h3"), 4: ("x4",)}
F_ORDER = ["h3", "h2", "x4"]        # outer -> inner in f


def roll_pieces(axis, shift):
    """Pieces of dst(x) = src(x + shift*e_axis) on the torus.

    Returns list of (ranges, deltas): ranges = {sub: (lo, hi)} dst-side box
    (unlisted subs full), deltas = {sub: d} with src_digit = dst_digit + d.
    """
    shift = shift % L
    if shift == 0:
        return [({}, {})]
    subs = AXIS_SUBS[axis]
    if len(subs) == 1:
        (M,) = subs
        R = SUB[M][0]
        return [
            ({M: (0, R - shift)}, {M: shift}),
            ({M: (R - shift, R)}, {M: shift - R}),
        ]
    Mj, mj = subs
    RM, Rm = SUB[Mj][0], SUB[mj][0]
    sM, sm = shift // Rm, shift % Rm
    pieces = []
    for carry, mlo, mhi, dm in (
        (sM, 0, Rm - sm, sm),
        (sM + 1, Rm - sm, Rm, sm - Rm),
    ):
        if mlo >= mhi:
            continue
        if carry == 0:
            pieces.append(({mj: (mlo, mhi)}, {mj: dm}))
        else:
            if RM - carry > 0:
                pieces.append(
                    ({mj: (mlo, mhi), Mj: (0, RM - carry)}, {mj: dm, Mj: carry})
                )
            pieces.append(
                ({mj: (mlo, mhi), Mj: (RM - carry, RM)}, {mj: dm, Mj: carry - RM})
            )
    out = []
    for rng, dl in pieces:
        rng = {k: v for k, v in rng.items() if v != (0, SUB[k][0])}
        dl = {k: v for k, v in dl.items() if v != 0}
        out.append((rng, dl))
    return out


def piece_geometry(rng, dl, extra_rng=None):
    """Geometry of one piece: (p_windows, f0, f_dims, runlen, dp, df).

    p_windows: [(dst_p_start, length)] contiguous partition windows.
    f0: dst f offset; f_dims: outer free dims [(stride, count)]; runlen:
    contiguous inner run length. dp/df: src = dst + delta.
    Returns None if the piece is empty after intersecting extra_rng.
    """
    rng = dict(rng)
    if extra_rng:
        for k, w in extra_rng.items():
            lo, hi = rng.get(k, (0, SUB[k][0]))
            lo, hi = max(lo, w[0]), min(hi, w[1])
            if lo >= hi:
                return None
            rng[k] = (lo, hi)

    def sub_rng(s):
        return rng.get(s, (0, SUB[s][0]))

    def sub_dl(s):
        return dl.get(s, 0)

    x1r, l2r, l3r = sub_rng("x1"), sub_rng("l2"), sub_rng("l3")
    p_windows = []
    if l2r == (0, 3) and l3r == (0, 3):
        p_windows = [(x1r[0] * 9, (x1r[1] - x1r[0]) * 9)]
    elif l3r == (0, 3):
        for x1 in range(*x1r):
            p_windows.append((x1 * 9 + l2r[0] * 3, (l2r[1] - l2r[0]) * 3))
    else:
        for x1 in range(*x1r):
            for l2 in range(*l2r):
                p_windows.append((x1 * 9 + l2 * 3 + l3r[0], l3r[1] - l3r[0]))
    dp = sub_dl("x1") * 9 + sub_dl("l2") * 3 + sub_dl("l3")
    df = sum(sub_dl(s) * SUB[s][2] for s in F_ORDER)

    f0 = 0
    run = 1
    extendable = True
    f_dims = []
    for s in reversed(F_ORDER):             # x4, h2, h3
        radix, _, fstr = SUB[s]
        lo, hi = sub_rng(s)
        n = hi - lo
        f0 += lo * fstr
        if extendable and run == fstr:
            run = fstr * n
            if n != radix:
                extendable = False
        else:
            extendable = False
            if n > 1:
                f_dims.insert(0, (fstr, n))
    return p_windows, f0, f_dims, run, dp, df


# ---------------------------------------------------------------------------
# host-side packing
# ---------------------------------------------------------------------------

def _site_perm():
    x = np.arange(NS).reshape(L, L, L, L)       # x1 x2 x3 x4
    x = x.reshape(L, 3, 4, 3, 4, L)             # x1 l2 h2 l3 h3 x4
    x = x.transpose(0, 1, 3, 4, 2, 5)           # x1 l2 l3 h3 h2 x4
    return x.reshape(-1)


_PERM = _site_perm()
_IPERM = np.argsort(_PERM)


def pack_u(U):
    """U [B,L,L,L,L,4,3,3] c64 -> [B, 72, NS] f32, comp=(dir, rj, elem)."""
    B = U.shape[0]
    Uf = U.reshape(B, NS, 4, 9)[:, _PERM]        # [B, NS(layout), 4, 9]
    out = np.empty((B, 4, 2, 9, NS), np.float32)
    out[:, :, 0] = Uf.real.transpose(0, 2, 3, 1)
    out[:, :, 1] = Uf.imag.transpose(0, 2, 3, 1)
    return np.ascontiguousarray(out.reshape(B, 72, NS))


def pack_omega(omega):
    """omega [8,10,4,3] c64 -> [20, 192] f32 lhsT; col=(mk, i, ri)."""
    O = np.zeros((10, 2, 12, N_OUT, 2), np.float32)
    for m in range(4):
        for kidx in range(3):
            om = np.asarray(omega[:, :, m, kidx])   # [i, j]
            mk = m * 3 + kidx
            O[:, 0, mk, :, 0] = om.real.T
            O[:, 1, mk, :, 0] = -om.imag.T
            O[:, 0, mk, :, 1] = om.imag.T
            O[:, 1, mk, :, 1] = om.real.T
    return np.ascontiguousarray(O.reshape(20, 192))


def unpack_out(raw):
    """raw [B, 144, NS] f32 (comp=(i,ri,elem)) -> [B,L,L,L,L,8,3,3] c64."""
    B = raw.shape[0]
    r = raw.reshape(B, N_OUT, 2, 9, NS)
    c = (r[:, :, 0] + 1j * r[:, :, 1]).astype(np.complex64)
    c = c.transpose(0, 3, 1, 2)[:, _IPERM]
    return np.ascontiguousarray(c.reshape(B, L, L, L, L, N_OUT, 3, 3))


# ---------------------------------------------------------------------------
# device program helpers
# ---------------------------------------------------------------------------

def _tile_pitch(tile):
    return tile[:].ap[0][0]


def _ftile(tile):
    """site-f size (innermost dim) of a [NP, C, F] tile."""
    return tile[:].ap[-1][1]


def _ap(tile, c0, f0, dims, np_=NP, p0=0):
    """Raw AP into pool tile [NP, C, Ftile]."""
    from concourse.bass import AP

    base = tile[:].offset
    pitch = _tile_pitch(tile)
    return AP(tile[:].tensor, base + p0 * pitch + c0 * _ftile(tile) + f0,
              [[pitch, np_]] + [list(d) for d in dims])


class CBlk:
    """Complex 3x3-matrix field block in an SBUF tile.

    tile [NP, C, Ftile]; comps are (rj*9 + 3*r + c) starting at c0; f window
    [f0, f0+nf) inside the tile.
    """

    def __init__(self, tile, c0=0, f0=0, nf=None):
        self.tile = tile
        self.c0 = c0
        self.f0 = f0
        self.nf = nf if nf is not None else _ftile(tile)

    def v(self, rj0, nrj, e0, ne, estride=1):
        Ft = _ftile(self.tile)
        dims = []
        if nrj > 1:
            dims.append((9 * Ft, nrj))
        if ne > 1:
            dims.append((estride * Ft, ne))
        dims.append((1, self.nf))
        return _ap(self.tile, self.c0 + rj0 * 9 + e0, self.f0, dims)


def cmul(nc, pool, out, A, B, adj_b=False, acc=False, gp_units=(), f32dt=None):
    """out = A @ B (complex 3x3 per site); adj_b: use B^dag; acc: out += ...

    A, B, out: CBlk with equal nf. gp_units: set of (3a+b) on gpsimd.
    """
    import concourse.mybir as mybir

    nf = out.nf
    for a in range(3):
        for b in range(3):
            on_gp = (3 * a + b) in gp_units
            eng = nc.gpsimd if on_gp else nc.vector
            sfx = "g" if on_gp else "v"
            m1 = pool.tile([NP, 6, nf], f32dt, tag=f"cm_m1{sfx}")
            m2 = pool.tile([NP, 3, nf], f32dt, tag=f"cm_m2{sfx}")
            m3 = pool.tile([NP, 3, nf], f32dt, tag=f"cm_m3{sfx}")
            t = pool.tile([NP, 1, nf], f32dt, tag=f"cm_t{sfx}")
            u = pool.tile([NP, 1, nf], f32dt, tag=f"cm_u{sfx}")
            v = pool.tile([NP, 1, nf], f32dt, tag=f"cm_v{sfx}")

            a_rr = A.v(0, 2, 3 * a, 3)
            if adj_b:
                b_rr = B.v(0, 2, 3 * b, 3)
                b_im = B.v(1, 1, 3 * b, 3)
                b_re = B.v(0, 1, 3 * b, 3)
            else:
                b_rr = B.v(0, 2, b, 3, estride=3)
                b_im = B.v(1, 1, b, 3, estride=3)
                b_re = B.v(0, 1, b, 3, estride=3)
            m1v = _ap(m1, 0, 0, [(3 * nf, 2), (nf, 3), (1, nf)])
            eng.tensor_tensor(out=m1v, in0=a_rr, in1=b_rr,
                              op=mybir.AluOpType.mult)
            m2v = _ap(m2, 0, 0, [(nf, 3), (1, nf)])
            eng.tensor_tensor(out=m2v, in0=A.v(0, 1, 3 * a, 3), in1=b_im,
                              op=mybir.AluOpType.mult)
            m3v = _ap(m3, 0, 0, [(nf, 3), (1, nf)])
            eng.tensor_tensor(out=m3v, in0=A.v(1, 1, 3 * a, 3), in1=b_re,
                              op=mybir.AluOpType.mult)

            def s3(dst_tile, src_tile, c_base):
                sv = lambda c: _ap(src_tile, c_base + c, 0, [(1, nf)])
                dv = _ap(dst_tile, 0, 0, [(1, nf)])
                eng.tensor_tensor(out=dv, in0=sv(0), in1=sv(1),
                                  op=mybir.AluOpType.add)
                eng.tensor_tensor(out=dv, in0=dv, in1=sv(2),
                                  op=mybir.AluOpType.add)
                return dv

            tv = s3(t, m1, 0)            # sum_c Ar*Br
            uv = s3(u, m1, 3)            # sum_c Ai*Bi
            cr_out = out.v(0, 1, 3 * a + b, 1)
            op_r = mybir.AluOpType.add if adj_b else mybir.AluOpType.subtract
            if acc:
                vv = _ap(v, 0, 0, [(1, nf)])
                eng.tensor_tensor(out=vv, in0=tv, in1=uv, op=op_r)
                eng.tensor_tensor(out=cr_out, in0=cr_out, in1=vv,
                                  op=mybir.AluOpType.add)
            else:
                eng.tensor_tensor(out=cr_out, in0=tv, in1=uv, op=op_r)

            tv = s3(t, m2, 0)            # sum_c Ar*Bi
            uv = s3(u, m3, 0)            # sum_c Ai*Br
            ci_out = out.v(1, 1, 3 * a + b, 1)
            op_i = mybir.AluOpType.subtract if adj_b else mybir.AluOpType.add
            if acc:
                vv = _ap(v, 0, 0, [(1, nf)])
                eng.tensor_tensor(out=vv, in0=uv, in1=tv, op=op_i)
                eng.tensor_tensor(out=ci_out, in0=ci_out, in1=vv,
                                  op=mybir.AluOpType.add)
            else:
                eng.tensor_tensor(out=ci_out, in0=uv, in1=tv, op=op_i)


def emit_roll(nc, dst_tile, dst_c0, ncomp, src, axis, shift, h3_win=None):
    """Fill dst_tile[:, dst_c0:+ncomp, :] with src rolled by shift along axis.

    src: ("sb", tile, c0) full-f SBUF tile, or ("dram", handle, comp0) with
    handle rows [*, NS] in layout order. h3_win: (lo, hi) h3 window; the dst
    tile then only covers f in [48*lo, 48*hi).
    """
    from concourse.bass import AP

    extra = {"h3": h3_win} if h3_win else None
    f_base = 48 * h3_win[0] if h3_win else 0
    for rng, dl in roll_pieces(axis, shift):
        geo = piece_geometry(rng, dl, extra)
        if geo is None:
            continue
        p_windows, f0, f_dims, run, dp, df = geo
        if dp == 0 and src[0] == "sb":
            s_tile, s_c0 = src[1], src[2]
            dims = [(_ftile(dst_tile), ncomp)] + f_dims + [(1, run)]
            sdims = [(_ftile(s_tile), ncomp)] + f_dims + [(1, run)]
            dv = _ap(dst_tile, dst_c0, f0 - f_base, dims)
            sv = _ap(s_tile, s_c0, f0 + df, sdims)
            nc.gpsimd.tensor_copy(out=dv, in_=sv)
            continue

        def emit_box(extra_f, dims_left):
            if dims_left:
                (st, cn), rest = dims_left[0], dims_left[1:]
                for k in range(cn):
                    emit_box(extra_f + k * st, rest)
                return
            for (p0, plen) in p_windows:
                dv = _ap(dst_tile, dst_c0, f0 + extra_f - f_base,
                         [(_ftile(dst_tile), ncomp), (1, run)],
                         np_=plen, p0=p0)
                if src[0] == "sb":
                    s_tile, s_c0 = src[1], src[2]
                    sv = _ap(s_tile, s_c0, f0 + extra_f + df,
                             [(_ftile(s_tile), ncomp), (1, run)],
                             np_=plen, p0=p0 + dp)
                else:
                    h, c0 = src[1], src[2]
                    off = c0 * NS + (p0 + dp) * NF + f0 + extra_f + df
                    sv = AP(h, off, [[NF, plen], [NS, ncomp], [1, run]])
                nc.sync.dma_start(out=dv, in_=sv)

        emit_box(0, f_dims)


PAIRS = [(1, 2), (1, 3), (1, 4), (2, 3), (2, 4), (3, 4)]
GP = {5, 8}           # cmul units routed to gpsimd
CS = 864              # phase C1 sites per chunk
NCOLS = 9 * CS        # 7776 = 486 * 16
SLAB = 486


def build_program(phases=("A", "C", "D")):
    import contextlib

    import concourse.mybir as mybir
    import concourse.tile as tile
    from concourse import bacc
    from concourse.bass import AP

    nc = bacc.Bacc("TRN2", target_bir_lowering=False, debug=False)
    f32 = mybir.dt.float32
    u_dram = nc.dram_tensor("u", [72, NS], f32, kind="ExternalInput")
    om_dram = nc.dram_tensor("om", [20, 192], f32, kind="ExternalInput")
    w_kind = "ExternalOutput" if "D" not in phases else "Internal"
    w_dram = nc.dram_tensor("w", [180, NS], f32, kind=w_kind)
    z_kind = "ExternalOutput" if phases == ("A", "C") else "Internal"
    z_dram = nc.dram_tensor("z", [192 * 9, NS], f32, kind=z_kind)
    out_dram = nc.dram_tensor("o", [144, NS], f32, kind="ExternalOutput")

    with tile.TileContext(nc) as tc:
        if "A" in phases:
            with tc.tile_pool(name="aone", bufs=1) as one, \
                 tc.tile_pool(name="awork", bufs=2) as work:
                _phase_a(nc, mybir, work, one, u_dram, w_dram)
        if "C" in phases:
            with tc.tile_pool(name="cone", bufs=1) as one, \
                 tc.tile_pool(name="cwork", bufs=2) as work, \
                 tc.tile_pool(name="psum", bufs=8, space="PSUM") as psum_pool:
                _phase_c1(nc, mybir, work, one, psum_pool, om_dram, w_dram,
                          z_dram)
        if "D" in phases:
            with tc.tile_pool(name="done", bufs=1) as one, \
                 tc.tile_pool(name="dwork", bufs=2) as work:
                _phase_c2d(nc, mybir, work, one, u_dram, z_dram, out_dram)
        if "D" not in phases:
            with tc.tile_pool(name="stub", bufs=1) as one:
                t = one.tile([1, 144], mybir.dt.float32, tag="zz")
                nc.vector.memset(t[:, :], 0.0)
                nc.sync.dma_start(out=AP(out_dram, 0, [[NS, 1], [1, 144]]),
                                  in_=t[:, :])
    nc.compile()
    return nc


def _phase_a(nc, mybir, work, one, u_dram, w_dram):
    from concourse.bass import AP

    f32 = mybir.dt.float32
    U_sb = one.tile([NP, 72, NF], f32, tag="U")
    nc.sync.dma_start(out=U_sb[:, :, :],
                      in_=AP(u_dram, 0, [[NF, NP], [NS, 72], [1, NF]]))

    def store_w(ch, t):
        nc.sync.dma_start(
            out=AP(w_dram, ch * 18 * NS, [[NF, NP], [NS, 18], [1, NF]]),
            in_=t[:, :, :])

    def wt(tag):
        return work.tile([NP, 18, NF], f32, tag=tag, name=tag)

    for ch, (mu, nu) in enumerate(PAIRS):
        rb1 = wt("Ta")
        emit_roll(nc, rb1, 0, 18, ("sb", U_sb, (nu - 1) * 18), mu, 1)
        rb2 = wt("Tb")
        emit_roll(nc, rb2, 0, 18, ("sb", U_sb, (mu - 1) * 18), nu, 1)
        m1t = wt("Tc")
        cmul(nc, work, CBlk(m1t), CBlk(U_sb, (mu - 1) * 18), CBlk(rb1),
             gp_units=GP, f32dt=f32)
        m2t = wt("Ta")
        cmul(nc, work, CBlk(m2t), CBlk(m1t), CBlk(rb2), adj_b=True,
             gp_units=GP, f32dt=f32)
        pt = wt("Tb")
        cmul(nc, work, CBlk(pt), CBlk(m2t), CBlk(U_sb, (nu - 1) * 18),
             adj_b=True, gp_units=GP, f32dt=f32)
        store_w(ch, pt)

    for d in range(1, 5):
        c0 = (d - 1) * 18
        r = wt("Ta")
        emit_roll(nc, r, 0, 18, ("sb", U_sb, c0), d, 1)
        p2 = wt("Tb")
        cmul(nc, work, CBlk(p2), CBlk(U_sb, c0), CBlk(r), gp_units=GP,
             f32dt=f32)
        r = wt("Tc")
        emit_roll(nc, r, 0, 18, ("sb", p2, 0), d, 2)
        p4 = wt("Td")
        cmul(nc, work, CBlk(p4), CBlk(p2), CBlk(r), gp_units=GP, f32dt=f32)
        r = wt("Ta")
        emit_roll(nc, r, 0, 18, ("sb", p4, 0), d, 4)
        p8 = wt("Tb")
        cmul(nc, work, CBlk(p8), CBlk(p4), CBlk(r), gp_units=GP, f32dt=f32)
        r = wt("Tc")
        emit_roll(nc, r, 0, 18, ("sb", p4, 0), d, 8)
        p12 = wt("Td")
        cmul(nc, work, CBlk(p12), CBlk(p8), CBlk(r), gp_units=GP, f32dt=f32)
        store_w(6 + d - 1, p12)


def _phase_c1(nc, mybir, work, one, psum_pool, om_dram, w_dram, z_dram):
    from concourse.bass import AP

    f32 = mybir.dt.float32
    om_sb = one.tile([20, 192], f32, tag="om")
    nc.sync.dma_start(out=om_sb[:, :], in_=om_dram.ap())
    for chunk in range(NS // CS):
        rhs = work.tile([20, NCOLS], f32, tag="rhs")
        nc.sync.dma_start(
            out=rhs[:, :],
            in_=AP(w_dram, chunk * CS, [[9 * NS, 20], [NS, 9], [1, CS]]))
        for hb in range(2):
            zst = work.tile([96, NCOLS], f32, tag=f"zst{hb}")
            for sl in range(NCOLS // SLAB):
                ps = psum_pool.tile([96, SLAB], f32, tag="ps")
                nc.tensor.matmul(
                    ps[:, :],
                    om_sb[:, 96 * hb:96 * (hb + 1)],
                    rhs[:, sl * SLAB:(sl + 1) * SLAB],
                    start=True, stop=True)
                nc.any.tensor_copy(out=zst[:, sl * SLAB:(sl + 1) * SLAB],
                                   in_=ps[:, :])
            nc.sync.dma_start(
                out=AP(z_dram, hb * 96 * 9 * NS + chunk * CS,
                       [[9 * NS, 96], [NS, 9], [1, CS]]),
                in_=zst[:, :])


def _phase_c2d(nc, mybir, work, one, u_dram, z_dram, out_dram):
    from concourse.bass import AP

    f32 = mybir.dt.float32
    for q in range(4):          # h3 quarters: f window [48q, 48q+48)
        uq = one.tile([NP, 72, 48], f32, tag="uq")
        nc.sync.dma_start(
            out=uq[:, :, :],
            in_=AP(u_dram, 48 * q, [[NF, NP], [NS, 72], [1, 48]]))
        oacc = one.tile([NP, 144, 48], f32, tag="oacc")
        for m in range(4):
            s_t = work.tile([NP, 144, 48], f32, tag="s")
            t0 = work.tile([NP, 144, 48], f32, tag="t0")
            emit_roll(nc, s_t, 0, 144, ("dram", z_dram, (m * 3 + 1) * 144),
                      m + 1, 0, h3_win=(q, q + 1))
            emit_roll(nc, t0, 0, 144, ("dram", z_dram, (m * 3 + 0) * 144),
                      m + 1, -1, h3_win=(q, q + 1))
            nc.vector.tensor_tensor(out=s_t[:, :, :], in0=s_t[:, :, :],
                                    in1=t0[:, :, :], op=mybir.AluOpType.add)
            t2 = work.tile([NP, 144, 48], f32, tag="t0")
            emit_roll(nc, t2, 0, 144, ("dram", z_dram, (m * 3 + 2) * 144),
                      m + 1, 1, h3_win=(q, q + 1))
            nc.vector.tensor_tensor(out=s_t[:, :, :], in0=s_t[:, :, :],
                                    in1=t2[:, :, :], op=mybir.AluOpType.add)
            um = CBlk(uq, m * 18)
            for i in range(N_OUT):
                tt = work.tile([NP, 18, 48], f32, tag="tt")
                cmul(nc, work, CBlk(tt), CBlk(s_t, i * 18), um, adj_b=True,
                     gp_units=GP, f32dt=f32)
                cmul(nc, work, CBlk(oacc, i * 18), um, CBlk(tt),
                     acc=(m > 0), gp_units=GP, f32dt=f32)
        nc.sync.dma_start(
            out=AP(out_dram, 48 * q, [[NF, NP], [NS, 144], [1, 48]]),
            in_=oacc[:, :, :])


# ---------------------------------------------------------------------------
# host runner
# ---------------------------------------------------------------------------

_CACHE = {}


def _get_nc():
    if "nc" not in _CACHE:
        _CACHE["nc"] = build_program()
    return _CACHE["nc"]


def kernel(U, omega, K=1, N_out=8, **_):
    U = np.asarray(U)
    omega = np.asarray(omega)
    B = U.shape[0]
    u_packed = pack_u(U)                      # [B, 72, NS]
    om = pack_omega(omega)                    # [20, 192]
    nc = _get_nc()
    from concourse.bass_utils import run_bass_kernel_spmd

    in_maps = [{"u": u_packed[b], "om": om} for b in range(B)]
    res = run_bass_kernel_spmd(nc, in_maps, core_ids=list(range(B)))
    raw = np.stack([r["o"] for r in res.results], axis=0)  # [B, 144, NS]
    return unpack_out(raw)


def time_device_exec(np_inputs, iters=3):
    """Repeat the sharded device execution with device-resident inputs and
    return per-iteration wall times of execute+sync only."""
    import time

    import jax
    import numpy as _np
    from jax.sharding import Mesh, NamedSharding, PartitionSpec
    from jax.experimental.shard_map import shard_map

    import concourse.mybir as mybir
    from concourse import bass2jax

    nc = _get_nc()
    bass2jax.install_neuronx_cc_hook()

    U = np.asarray(np_inputs["U"])
    omega = np.asarray(np_inputs["omega"])
    B = U.shape[0]
    u_packed = pack_u(U)
    om = pack_omega(omega)
    in_maps = [{"u": u_packed[b], "om": om} for b in range(B)]

    partition_name = (nc.partition_id_tensor.name
                      if nc.partition_id_tensor else None)
    in_names, out_names, out_avals, zero_outs = [], [], [], []
    for alloc in nc.m.functions[0].allocations:
        if not isinstance(alloc, mybir.MemoryLocationSet):
            continue
        name = alloc.memorylocations[0].name
        if alloc.kind == "ExternalInput":
            if name != partition_name:
                in_names.append(name)
        elif alloc.kind == "ExternalOutput":
            shape = tuple(alloc.tensor_shape)
            dtype = mybir.dt.np(alloc.dtype)
            out_names.append(name)
            out_avals.append(jax.core.ShapedArray(shape, dtype))
            zero_outs.append(_np.zeros(shape, dtype))
    n_params = len(in_names)
    all_in_names = list(in_names) + list(out_names)
    if partition_name is not None:
        all_in_names.append(partition_name)

    def _body(*args):
        operands = list(args)
        if partition_name is not None:
            operands.append(bass2jax.partition_id_tensor())
        outs = bass2jax._bass_exec_p.bind(
            *operands,
            out_avals=tuple(out_avals),
            in_names=tuple(all_in_names),
            out_names=tuple(out_names),
            lowering_input_output_aliases=(),
            sim_require_finite=True,
            sim_require_nnan=True,
            nc=nc,
        )
        return tuple(outs)

    devices = jax.devices()[:B]
    mesh = Mesh(_np.asarray(devices), ("core",))
    nspec = (PartitionSpec("core"),)
    fn = jax.jit(shard_map(_body, mesh=mesh,
                           in_specs=nspec * (n_params + len(out_names)),
                           out_specs=nspec * len(out_names),
                           check_rep=False))
    sh = NamedSharding(mesh, PartitionSpec("core"))
    dev_args = []
    for i, name in enumerate(in_names):
        cat = _np.concatenate([in_maps[c][name] for c in range(B)], axis=0)
        dev_args.append(jax.device_put(cat, sh))
    for z in zero_outs:
        cat = _np.zeros((B * z.shape[0],) + z.shape[1:], z.dtype)
        dev_args.append(jax.device_put(cat, sh))

    outs = fn(*dev_args)          # warmup / compile
    jax.block_until_ready(outs)
    times = []
    for _ in range(iters):
        t0 = time.perf_counter()
        outs = fn(*dev_args)
        jax.block_until_ready(outs)
        times.append(time.perf_counter() - t0)
    return times
